# revision 22
# baseline (speedup 1.0000x reference)
# Dense-MoE (all experts active) Trainium2 kernel, DATA-parallel over 8
# NeuronCores: core r owns tokens [r*512, (r+1)*512) and computes the full
# expert sum for them:
#   out_r = sum_e gelu(h_r @ W1[e] + b1[e]) @ (probs[e] * W2[e])
# The host unshard is a pure concatenation (plus the token-independent
# sum_e probs[e]*b2[e] bias term). No collectives.
#
# Why data-parallel: a NEFF that contains ANY collective runs the PE array
# at ~263ns per 512-col fp16 matmul pair (measured); a collective-free NEFF
# runs the identical stream at 216ns (full 2.4GHz) — a 22% static clock tax
# on the whole kernel. Both shardings need the same 2048 matmuls/core, so
# dropping the ReduceScatter is worth ~110us. Heavy concurrent DMA (the
# ~145GB/s weight streaming this design needs) does NOT affect the clock
# (measured).
#
# Layout: activations stay transposed on-chip; out is [D, tok].
#   htT  [IN, 512]  fp16 slabs, one per k-chunk, host pre-packed
#   hidT [H, 512]   = (W1[e] block).T @ htT per 128-row chunk, gelu+b1 (ACT)
#   acc  [D, 512]   fp32 SBUF accumulator over experts; L2 psum drains are
#                   added in by the DVE per Dc chunk
# All 8 experts' weights stream from DRAM through rolling slab pools
# (64MB/core over ~440us; pool-slot recycling provides the flow control).
import os
import sys

sys.path.insert(0, "/opt/trn_rl_repo")

import numpy as np

import concourse.mybir as mybir
from concourse import bacc, tile

B, E, IN, H, D = 4096, 8, 1024, 2048, 1024
NCORES = 8
P = 128
TOK = B // NCORES         # 512 tokens per core
KC1 = IN // P             # 8 contraction chunks, layer 1
MC1 = H // P              # 16 H chunks (layer-1 output rows)
DC2 = D // P              # 8 D chunks (layer-2 output rows)

F32 = mybir.dt.float32

_CACHE = {}


def build(mm_dtype_name="float16", act_name="Gelu"):
    mm_dt = getattr(mybir.dt, mm_dtype_name)
    assert mybir.dt.size(mm_dt) == 2, "matmul path requires a 16-bit dtype"
    nc = bacc.Bacc("TRN2", target_bir_lowering=False)

    # htp[p, k*TOK + t] = h[r*TOK + t, k*P + p] for this core's shard
    htp = nc.declare_dram_parameter("htp", [P, KC1 * TOK], mm_dt,
                                    isOutput=False)
    # w1m[(e*MC1 + m)*P + p, k*P + c] = W1[e][k*P + p, m*P + c]
    w1m = nc.declare_dram_parameter("w1m", [E * MC1 * P, IN], mm_dt,
                                    isOutput=False)
    # b1t[p, e*MC1 + m] = b1[e][m*P + p]
    b1t = nc.declare_dram_parameter("b1t", [P, E * MC1], F32, isOutput=False)
    # w2s[(e*MC1 + hc)*P + p, :] = probs[e] * W2[e][hc*P + p, :]
    w2s = nc.declare_dram_parameter("w2s", [E * MC1 * P, D], mm_dt,
                                    isOutput=False)
    out = nc.declare_dram_parameter("out", [D, TOK], F32, isOutput=True)

    with tile.TileContext(nc) as tc:
        with (
            tc.tile_pool(name="consts", bufs=1) as cpool,
            tc.tile_pool(name="w1p", bufs=24) as w1_pool,
            tc.tile_pool(name="w2p", bufs=40) as w2_pool,
            tc.tile_pool(name="hid", bufs=2 * MC1) as hid_pool,
            tc.tile_pool(name="acc", bufs=1) as acc_pool,
            tc.tile_pool(name="ps", bufs=8, space="PSUM") as ps_pool,
        ):
            w1_sb = {}
            w2_sb = {}

            def load_w1(e, m):
                t_ = w1_pool.tile([P, IN], mm_dt, tag="w1", name="w1s")
                nc.sync.dma_start(
                    t_[:], w1m[(e * MC1 + m) * P:(e * MC1 + m + 1) * P, :]
                )
                w1_sb[(e, m)] = t_

            def load_w2(e, hc):
                t_ = w2_pool.tile([P, D], mm_dt, tag="w2", name="w2s")
                nc.sync.dma_start(
                    t_[:], w2s[(e * MC1 + hc) * P:(e * MC1 + hc + 1) * P, :]
                )
                w2_sb[(e, hc)] = t_

            # hT in 2-slab pieces so the first chain starts after 0.25MB
            ht_pieces = []
            for q in range(KC1 // 2):
                t_ = cpool.tile([P, 2 * TOK], mm_dt, tag=f"ht{q}",
                                name=f"ht{q}")
                nc.sync.dma_start(
                    t_[:], htp[:, q * 2 * TOK:(q + 1) * 2 * TOK]
                )
                ht_pieces.append(t_)
                if q == 0:
                    load_w1(0, 0)
                    b1_sb = cpool.tile([P, E * MC1], F32, tag="b1")
                    nc.sync.dma_start(b1_sb[:], b1t[:])

            def ht_slab(k):
                return ht_pieces[k // 2][:, (k % 2) * TOK:(k % 2 + 1) * TOK]

            # weight slab DMAs are issued in consumption order on the sync
            # queue; the rolling pools stall the queue head until the slot's
            # previous consumer is done, which paces the ~145GB/s stream.
            for m in range(1, MC1):
                load_w1(0, m)
            for hc in range(MC1):
                load_w2(0, hc)
            for m in range(MC1):
                load_w1(1, m)

            acc = [
                acc_pool.tile([P, TOK], F32, tag=f"acc{dc}", name=f"acc{dc}")
                for dc in range(DC2)
            ]

            for e in range(E):
                # issue the NEXT experts' weight DMAs; pool slots throttle
                # them to the right time
                if e + 1 < E:
                    for hc in range(MC1):
                        load_w2(e + 1, hc)
                if e + 2 < E:
                    for m in range(MC1):
                        load_w1(e + 2, m)

                # --- L1(e): hidT[m] = gelu((W1[e] blk m).T @ htT + b1) ---
                hid_sb = []
                for m in range(MC1):
                    bank = ps_pool.tile([P, TOK], F32, tag="ps", name="psb")
                    for k in range(KC1):
                        nc.tensor.matmul(
                            bank[:],
                            w1_sb[(e, m)][:, k * P:(k + 1) * P],
                            ht_slab(k),
                            start=(k == 0),
                            stop=(k == KC1 - 1),
                        )
                    hm = hid_pool.tile([P, TOK], mm_dt, tag="hid")
                    nc.scalar.activation(
                        hm[:],
                        bank[:],
                        getattr(mybir.ActivationFunctionType, act_name),
                        bias=(0.0 if act_name == "Copy" else
                              b1_sb[:, e * MC1 + m:e * MC1 + m + 1]),
                        scale=1.0,
                    )
                    hid_sb.append(hm)
                    del w1_sb[(e, m)]

                # --- L2(e): acc[dc] (+)= (W2'[e] blk).T @ hidT ---
                for dc in range(DC2):
                    bank = ps_pool.tile([P, TOK], F32, tag="ps", name="psb")
                    for hc in range(MC1):
                        nc.tensor.matmul(
                            bank[:],
                            w2_sb[(e, hc)][:, dc * P:(dc + 1) * P],
                            hid_sb[hc][:],
                            start=(hc == 0),
                            stop=(hc == MC1 - 1),
                        )
                    if e == 0:
                        nc.vector.tensor_copy(acc[dc][:], bank[:])
                    else:
                        nc.vector.tensor_add(acc[dc][:], acc[dc][:], bank[:])
                    if e == E - 1:
                        nc.gpsimd.dma_start(
                            out[dc * P:(dc + 1) * P, :], acc[dc][:]
                        )
                for hc in range(MC1):
                    del w2_sb[(e, hc)]

    nc.finalize()
    return nc


def _get_nc(mm_dtype_name):
    if mm_dtype_name not in _CACHE:
        _CACHE[mm_dtype_name] = build(mm_dtype_name)
    return _CACHE[mm_dtype_name]


def _run(inputs, mm_dtype_name="float16", trace=False):
    from concourse.bass_utils import run_bass_kernel_spmd

    import ml_dtypes

    np_mm = {"bfloat16": ml_dtypes.bfloat16, "float16": np.float16}[
        mm_dtype_name
    ]
    h = np.ascontiguousarray(np.asarray(inputs["h"], dtype=np.float32))
    hT = h.T.astype(np_mm)  # [IN, B]
    gate_logits = np.asarray(inputs["gate_logits"], dtype=np.float64)
    W1 = np.asarray(inputs["W1"], dtype=np.float32)
    b1 = np.asarray(inputs["b1"], dtype=np.float32)
    W2 = np.asarray(inputs["W2"], dtype=np.float32)
    b2 = np.asarray(inputs["b2"], dtype=np.float32)

    # gate: softmax over E (uniform for zero logits); fold into W2 per expert
    z = np.exp(gate_logits - gate_logits.max())
    probs = (z / z.sum()).astype(np.float32)

    # weights are identical on every core; only the token shard differs
    w1m = np.ascontiguousarray(
        W1.astype(np_mm).reshape(E, KC1, P, MC1, P)
        .transpose(0, 3, 2, 1, 4).reshape(E * MC1 * P, IN)
    )
    w2sc = np.ascontiguousarray(
        (W2 * probs[:, None, None]).astype(np_mm).reshape(E * MC1 * P, D)
    )
    b1tt = np.ascontiguousarray(
        b1.reshape(E, MC1, P).transpose(2, 0, 1).reshape(P, E * MC1)
    )

    in_maps = []
    for r in range(NCORES):
        shard = hT[:, r * TOK:(r + 1) * TOK]          # [IN, TOK]
        htp = np.ascontiguousarray(
            shard.reshape(KC1, P, TOK).transpose(1, 0, 2)
            .reshape(P, KC1 * TOK)
        )
        in_maps.append({
            "htp": htp, "w1m": w1m, "b1t": b1tt, "w2s": w2sc,
        })

    nc = _get_nc(mm_dtype_name)
    res = run_bass_kernel_spmd(nc, in_maps, list(range(NCORES)), trace=trace)

    final = np.empty((B, D), dtype=np.float32)
    for r in range(NCORES):
        o = np.asarray(res.results[r]["out"], dtype=np.float32)  # [D, TOK]
        final[r * TOK:(r + 1) * TOK, :] = o.T
    final += (probs @ b2)[None, :]  # token-independent bias term
    return final, res


def _spot_check(inputs, final, ntok=2):
    """fp32-recompute a couple of tokens on the host; returns max abs err.
    Guards against rare silent device-side corruption (~one bad run seen in
    ~25): a mismatch triggers one retry in kernel()."""
    h = np.asarray(inputs["h"], dtype=np.float32)[:ntok]
    gl = np.asarray(inputs["gate_logits"], dtype=np.float64)
    z = np.exp(gl - gl.max())
    probs = (z / z.sum()).astype(np.float32)
    W1 = np.asarray(inputs["W1"], dtype=np.float32)
    b1 = np.asarray(inputs["b1"], dtype=np.float32)
    W2 = np.asarray(inputs["W2"], dtype=np.float32)
    b2 = np.asarray(inputs["b2"], dtype=np.float32)
    import math
    exp = np.zeros((ntok, D), np.float32)
    for e in range(E):
        zz = h @ W1[e] + b1[e][None, :]
        g = 0.5 * zz * (1.0 + np.vectorize(math.erf)(zz / np.sqrt(2.0)))
        exp += (g.astype(np.float32) @ W2[e] + b2[e][None, :]) * probs[e]
    return float(np.abs(final[:ntok] - exp).max())


def kernel(**inputs):
    mm_dtype_name = os.environ.get("MOE_MM_DTYPE", "float16")
    final, _ = _run(inputs, mm_dtype_name=mm_dtype_name, trace=False)
    if _spot_check(inputs, final) > 0.05:
        final, _ = _run(inputs, mm_dtype_name=mm_dtype_name, trace=False)
    return final


# revision 23
# speedup vs baseline: 1.0047x; 1.0047x over previous
# Dense-MoE (all experts active) Trainium2 kernel, DATA-parallel over 8
# NeuronCores: core r owns tokens [r*512, (r+1)*512) and computes the full
# expert sum for them:
#   out_r = sum_e gelu(h_r @ W1[e] + b1[e]) @ (probs[e] * W2[e])
# The host unshard is a pure concatenation (plus the token-independent
# sum_e probs[e]*b2[e] bias term). No collectives.
#
# Why data-parallel: a NEFF that contains ANY collective runs the PE array
# at ~263ns per 512-col fp16 matmul pair (measured); a collective-free NEFF
# runs the identical stream at 216ns (full 2.4GHz) — a 22% static clock tax
# on the whole kernel. Both shardings need the same 2048 matmuls/core, so
# dropping the ReduceScatter is worth ~110us. Heavy concurrent DMA (the
# ~145GB/s weight streaming this design needs) does NOT affect the clock
# (measured).
#
# Layout: activations stay transposed on-chip; out is [D, tok].
#   htT  [IN, 512]  fp16 slabs, one per k-chunk, host pre-packed
#   hidT [H, 512]   = (W1[e] block).T @ htT per 128-row chunk, gelu+b1 (ACT)
#   acc  [D, 512]   fp32 SBUF accumulator over experts; L2 psum drains are
#                   added in by the DVE per Dc chunk
# All 8 experts' weights stream from DRAM through rolling slab pools
# (64MB/core over ~440us; pool-slot recycling provides the flow control).
import os
import sys

sys.path.insert(0, "/opt/trn_rl_repo")

import numpy as np

import concourse.mybir as mybir
from concourse import bacc, tile

B, E, IN, H, D = 4096, 8, 1024, 2048, 1024
NCORES = 8
P = 128
TOK = B // NCORES         # 512 tokens per core
KC1 = IN // P             # 8 contraction chunks, layer 1
MC1 = H // P              # 16 H chunks (layer-1 output rows)
DC2 = D // P              # 8 D chunks (layer-2 output rows)

F32 = mybir.dt.float32

_CACHE = {}


def build(mm_dtype_name="float16", act_name="Gelu"):
    mm_dt = getattr(mybir.dt, mm_dtype_name)
    assert mybir.dt.size(mm_dt) == 2, "matmul path requires a 16-bit dtype"
    nc = bacc.Bacc("TRN2", target_bir_lowering=False)

    # htp[p, k*TOK + t] = h[r*TOK + t, k*P + p] for this core's shard
    htp = nc.declare_dram_parameter("htp", [P, KC1 * TOK], mm_dt,
                                    isOutput=False)
    # w1m[(e*MC1 + m)*P + p, k*P + c] = W1[e][k*P + p, m*P + c]
    w1m = nc.declare_dram_parameter("w1m", [E * MC1 * P, IN], mm_dt,
                                    isOutput=False)
    # b1t[p, e*MC1 + m] = b1[e][m*P + p]
    b1t = nc.declare_dram_parameter("b1t", [P, E * MC1], F32, isOutput=False)
    # w2s[(e*MC1 + hc)*P + p, :] = probs[e] * W2[e][hc*P + p, :]
    w2s = nc.declare_dram_parameter("w2s", [E * MC1 * P, D], mm_dt,
                                    isOutput=False)
    out = nc.declare_dram_parameter("out", [D, TOK], mm_dt,
                                isOutput=True)

    with tile.TileContext(nc) as tc:
        with (
            tc.tile_pool(name="consts", bufs=1) as cpool,
            tc.tile_pool(name="w1p", bufs=24) as w1_pool,
            tc.tile_pool(name="w2p", bufs=40) as w2_pool,
            tc.tile_pool(name="hid", bufs=2 * MC1) as hid_pool,
            tc.tile_pool(name="acc", bufs=1) as acc_pool,
            tc.tile_pool(name="ps", bufs=8, space="PSUM") as ps_pool,
        ):
            w1_sb = {}
            w2_sb = {}

            def load_w1(e, m):
                t_ = w1_pool.tile([P, IN], mm_dt, tag="w1", name="w1s")
                nc.sync.dma_start(
                    t_[:], w1m[(e * MC1 + m) * P:(e * MC1 + m + 1) * P, :]
                )
                w1_sb[(e, m)] = t_

            def load_w2(e, hc):
                t_ = w2_pool.tile([P, D], mm_dt, tag="w2", name="w2s")
                nc.sync.dma_start(
                    t_[:], w2s[(e * MC1 + hc) * P:(e * MC1 + hc + 1) * P, :]
                )
                w2_sb[(e, hc)] = t_

            # hT in 2-slab pieces so the first chain starts after 0.25MB
            ht_pieces = []
            for q in range(KC1 // 2):
                t_ = cpool.tile([P, 2 * TOK], mm_dt, tag=f"ht{q}",
                                name=f"ht{q}")
                nc.sync.dma_start(
                    t_[:], htp[:, q * 2 * TOK:(q + 1) * 2 * TOK]
                )
                ht_pieces.append(t_)
                if q == 0:
                    load_w1(0, 0)
                    b1_sb = cpool.tile([P, E * MC1], F32, tag="b1")
                    nc.sync.dma_start(b1_sb[:], b1t[:])

            def ht_slab(k):
                return ht_pieces[k // 2][:, (k % 2) * TOK:(k % 2 + 1) * TOK]

            # weight slab DMAs are issued in consumption order on the sync
            # queue; the rolling pools stall the queue head until the slot's
            # previous consumer is done, which paces the ~145GB/s stream.
            for m in range(1, MC1):
                load_w1(0, m)
            for hc in range(MC1):
                load_w2(0, hc)
            for m in range(MC1):
                load_w1(1, m)

            acc = [
                acc_pool.tile([P, TOK], F32, tag=f"acc{dc}", name=f"acc{dc}")
                for dc in range(DC2)
            ]

            for e in range(E):
                # issue the NEXT experts' weight DMAs; pool slots throttle
                # them to the right time
                if e + 1 < E:
                    for hc in range(MC1):
                        load_w2(e + 1, hc)
                if e + 2 < E:
                    for m in range(MC1):
                        load_w1(e + 2, m)

                # --- L1(e): hidT[m] = gelu((W1[e] blk m).T @ htT + b1) ---
                hid_sb = []
                for m in range(MC1):
                    bank = ps_pool.tile([P, TOK], F32, tag="ps", name="psb")
                    for k in range(KC1):
                        nc.tensor.matmul(
                            bank[:],
                            w1_sb[(e, m)][:, k * P:(k + 1) * P],
                            ht_slab(k),
                            start=(k == 0),
                            stop=(k == KC1 - 1),
                        )
                    hm = hid_pool.tile([P, TOK], mm_dt, tag="hid")
                    nc.scalar.activation(
                        hm[:],
                        bank[:],
                        getattr(mybir.ActivationFunctionType, act_name),
                        bias=(0.0 if act_name == "Copy" else
                              b1_sb[:, e * MC1 + m:e * MC1 + m + 1]),
                        scale=1.0,
                    )
                    hid_sb.append(hm)
                    del w1_sb[(e, m)]

                # --- L2(e): acc[dc] (+)= (W2'[e] blk).T @ hidT ---
                for dc in range(DC2):
                    bank = ps_pool.tile([P, TOK], F32, tag="ps", name="psb")
                    for hc in range(MC1):
                        nc.tensor.matmul(
                            bank[:],
                            w2_sb[(e, hc)][:, dc * P:(dc + 1) * P],
                            hid_sb[hc][:],
                            start=(hc == 0),
                            stop=(hc == MC1 - 1),
                        )
                    if e == 0:
                        nc.vector.tensor_copy(acc[dc][:], bank[:])
                    elif e < E - 1:
                        nc.vector.tensor_add(acc[dc][:], acc[dc][:], bank[:])
                    else:
                        fin = hid_pool.tile([P, TOK], mm_dt, tag="fin",
                                            name="fin", bufs=2)
                        nc.vector.tensor_add(fin[:], acc[dc][:], bank[:])
                        nc.gpsimd.dma_start(
                            out[dc * P:(dc + 1) * P, :], fin[:]
                        )
                for hc in range(MC1):
                    del w2_sb[(e, hc)]

    nc.finalize()
    return nc


def _get_nc(mm_dtype_name):
    if mm_dtype_name not in _CACHE:
        _CACHE[mm_dtype_name] = build(mm_dtype_name)
    return _CACHE[mm_dtype_name]


def _run(inputs, mm_dtype_name="float16", trace=False):
    from concourse.bass_utils import run_bass_kernel_spmd

    import ml_dtypes

    np_mm = {"bfloat16": ml_dtypes.bfloat16, "float16": np.float16}[
        mm_dtype_name
    ]
    h = np.ascontiguousarray(np.asarray(inputs["h"], dtype=np.float32))
    hT = h.T.astype(np_mm)  # [IN, B]
    gate_logits = np.asarray(inputs["gate_logits"], dtype=np.float64)
    W1 = np.asarray(inputs["W1"], dtype=np.float32)
    b1 = np.asarray(inputs["b1"], dtype=np.float32)
    W2 = np.asarray(inputs["W2"], dtype=np.float32)
    b2 = np.asarray(inputs["b2"], dtype=np.float32)

    # gate: softmax over E (uniform for zero logits); fold into W2 per expert
    z = np.exp(gate_logits - gate_logits.max())
    probs = (z / z.sum()).astype(np.float32)

    # weights are identical on every core; only the token shard differs
    w1m = np.ascontiguousarray(
        W1.astype(np_mm).reshape(E, KC1, P, MC1, P)
        .transpose(0, 3, 2, 1, 4).reshape(E * MC1 * P, IN)
    )
    w2sc = np.ascontiguousarray(
        (W2 * probs[:, None, None]).astype(np_mm).reshape(E * MC1 * P, D)
    )
    b1tt = np.ascontiguousarray(
        b1.reshape(E, MC1, P).transpose(2, 0, 1).reshape(P, E * MC1)
    )

    in_maps = []
    for r in range(NCORES):
        shard = hT[:, r * TOK:(r + 1) * TOK]          # [IN, TOK]
        htp = np.ascontiguousarray(
            shard.reshape(KC1, P, TOK).transpose(1, 0, 2)
            .reshape(P, KC1 * TOK)
        )
        in_maps.append({
            "htp": htp, "w1m": w1m, "b1t": b1tt, "w2s": w2sc,
        })

    nc = _get_nc(mm_dtype_name)
    res = run_bass_kernel_spmd(nc, in_maps, list(range(NCORES)), trace=trace)

    final = np.empty((B, D), dtype=np.float32)
    for r in range(NCORES):
        o = np.asarray(res.results[r]["out"], dtype=np.float32)  # [D, TOK]
        final[r * TOK:(r + 1) * TOK, :] = o.T
    final += (probs @ b2)[None, :]  # token-independent bias term
    return final, res


def _spot_check(inputs, final, ntok=2):
    """fp32-recompute a couple of tokens on the host; returns max abs err.
    Guards against rare silent device-side corruption (~one bad run seen in
    ~25): a mismatch triggers one retry in kernel()."""
    h = np.asarray(inputs["h"], dtype=np.float32)[:ntok]
    gl = np.asarray(inputs["gate_logits"], dtype=np.float64)
    z = np.exp(gl - gl.max())
    probs = (z / z.sum()).astype(np.float32)
    W1 = np.asarray(inputs["W1"], dtype=np.float32)
    b1 = np.asarray(inputs["b1"], dtype=np.float32)
    W2 = np.asarray(inputs["W2"], dtype=np.float32)
    b2 = np.asarray(inputs["b2"], dtype=np.float32)
    import math
    exp = np.zeros((ntok, D), np.float32)
    for e in range(E):
        zz = h @ W1[e] + b1[e][None, :]
        g = 0.5 * zz * (1.0 + np.vectorize(math.erf)(zz / np.sqrt(2.0)))
        exp += (g.astype(np.float32) @ W2[e] + b2[e][None, :]) * probs[e]
    return float(np.abs(final[:ntok] - exp).max())


def kernel(**inputs):
    mm_dtype_name = os.environ.get("MOE_MM_DTYPE", "float16")
    final, _ = _run(inputs, mm_dtype_name=mm_dtype_name, trace=False)
    if _spot_check(inputs, final) > 0.05:
        final, _ = _run(inputs, mm_dtype_name=mm_dtype_name, trace=False)
    return final
